# revision 38
# baseline (speedup 1.0000x reference)
"""DigitCaps dynamic-routing kernel for Trainium2 (8 NeuronCores, SPMD).

Problem:  u = einsum('bri,rcio->brco', x, W[0]);  3 routing iterations
          (softmax over capsules, weighted sum over routes, squash,
          agreement update);  returns v [B, C, OC].

Shapes: B=256, R=1152, C=10, IC=8, OC=16.  Batch-sharded 8 ways (BL=32
per core, zero cross-core communication).

Design notes (per core) -- batch-group-major software pipeline:
 - The 32 batches are processed as 4 groups of 8 (bg).  Routing is
   per-batch independent, so each bg's chain (u-phase -> it1 -> it2 ->
   it3 -> output) only depends on its own u.  The u-phase for bg1..3
   (tensor engine) therefore overlaps the routing iterations of earlier
   groups (vector engine), hiding most of the u-phase behind the DVE
   critical path instead of serializing phase-by-phase.
 - u-phase per bg: one matmul per 16-route chunk; lhsT is block-diag x
   ([128=(g,i), 128=(b8,g16)]), rhs the W chunk [128=(g,i), 160=(c,o)],
   fp32 PSUM, evacuated to resident bf16 u1 by the scalar engine only
   (the DVE never touches u-phase data: it is the kernel bottleneck).
 - Iter-1 shortcut: c is uniform 1/10, so s1 = 0.1*sum_r u via one
   extra accumulating matmul per chunk (plain x as lhsT, 8 rows per bg).
 - b-update (delta = sum_o u*v) is DVE mult (bf16 2x) + reduce (1x
   cap); softmax sum/reciprocal on DVE, exp on scalar.  These are the
   only DVE ops in steady state, so the pipeline is paced by them.
 - squash runs on scalar+gpsimd only (gp divide for the normalize), so
   per-bg squashes never head-of-line-block the DVE queue.
 - s-pass (s = sum_r c*u) on the tensor engine: lhsT is block-diag c
   built by a gpsimd mask-multiply, PSUM-accumulated over all 72
   chunks; the (c,c') diagonal is extracted with small DMAs.
 - Outputs stream out per bg (4 small DMAs) so the tail is short.
"""

import sys

sys.path.insert(0, "/opt/trn_rl_repo")

from contextlib import ExitStack

import ml_dtypes
import numpy as np

import concourse.bass as bass
import concourse.tile as tile
from concourse import bacc, mybir
from concourse.bass_utils import run_bass_kernel_spmd

BF16 = mybir.dt.bfloat16
F32 = mybir.dt.float32
AF = mybir.ActivationFunctionType
ALU = mybir.AluOpType
AX = mybir.AxisListType

B, R, C, IC, OC = 256, 1152, 10, 8, 16
NCORES = 8
BL = B // NCORES  # 32 batches per core
G = 16  # routes per chunk
NBG = BL // 8  # 4 b-groups of 8
CO = C * OC  # 160
EPS = 1e-8
NPBF = ml_dtypes.bfloat16

# Set by tests to shrink the problem for simulation; full size by default.
_R_OVERRIDE = None


def _nchunks(r=None):
    r = r if r is not None else (_R_OVERRIDE or R)
    assert r % G == 0
    return r // G


def _squash_sg(nc, pool, s_ap, v_ap, pre, np_=8):
    """v = squash(pre*s), DVE-centric: the ops are tiny ([8,C]-ish) and
    call sites defer emission by one bg-unit so they slot into the DVE
    queue right after the following group's delta chain instead of
    blocking it.  Only the sqrt leaves the DVE (one scalar-engine hop) --
    cross-engine ping-pong latency, not op cost, is what hurt here.

    With q = sum_o s^2 (unscaled), the scale factor is
      sc = pre^3*q / ((pre*sqrt(q) + EPS) * (1 + pre^2*q)),   v = sc * s.

    Uses sqrt(n2 + EPS^2) ~= nrm + EPS (difference is O(EPS) absolute,
    only relevant when the norm itself is ~EPS).
    """
    sq = pool.tile([np_, CO], F32, tag="sq")
    if s_ap.space == bass.MemorySpace.PSUM:
        # a PSUM tensor may be read only once per instruction
        nc.scalar.square(sq[:], s_ap)
    else:
        nc.vector.tensor_tensor(out=sq[:], in0=s_ap, in1=s_ap, op=ALU.mult)
    q = pool.tile([np_, C], F32, tag="n2")
    nc.vector.reduce_sum(
        out=q[:], in_=sq[:].rearrange("p (c o) -> p c o", c=C), axis=AX.X
    )
    nrm = pool.tile([np_, C], F32, tag="nrm")
    nc.scalar.sqrt(nrm[:], q[:])
    t1 = pool.tile([np_, C], F32, tag="t1")
    nc.vector.tensor_scalar(
        out=t1[:], in0=q[:], scalar1=pre * pre, scalar2=1.0,
        op0=ALU.mult, op1=ALU.add,
    )
    den = pool.tile([np_, C], F32, tag="den")
    nc.vector.tensor_scalar(
        out=den[:], in0=nrm[:], scalar1=pre, scalar2=EPS,
        op0=ALU.mult, op1=ALU.add,
    )
    nc.vector.tensor_tensor(out=den[:], in0=den[:], in1=t1[:], op=ALU.mult)
    rden = pool.tile([np_, C], F32, tag="rden")
    nc.vector.reciprocal(rden[:], den[:])
    sc = pool.tile([np_, C], F32, tag="sc")
    nc.vector.scalar_tensor_tensor(
        out=sc[:], in0=q[:], scalar=pre * pre * pre, in1=rden[:],
        op0=ALU.mult, op1=ALU.mult,
    )
    nc.vector.tensor_tensor(
        out=v_ap.rearrange("p (c o) -> p c o", c=C),
        in0=s_ap.rearrange("p (c o) -> p c o", c=C),
        in1=sc[:].unsqueeze(2).broadcast_to([np_, C, OC]),
        op=ALU.mult,
    )


def _body(ctx, tc, xbd_d, wt_d, xp_d, sel_d, msk_d, out_d, K):
    nc = tc.nc

    per = ctx.enter_context(tc.tile_pool(name="per", bufs=1))
    xbdp = ctx.enter_context(tc.tile_pool(name="xbdp", bufs=4))
    upsum = ctx.enter_context(tc.tile_pool(name="upsum", bufs=3, space="PSUM"))
    s1psum = ctx.enter_context(tc.tile_pool(name="s1psum", bufs=1, space="PSUM"))
    spsum = ctx.enter_context(tc.tile_pool(name="spsum", bufs=2, space="PSUM"))
    vbpsum = ctx.enter_context(tc.tile_pool(name="vbpsum", bufs=2, space="PSUM"))
    tmpp = ctx.enter_context(tc.tile_pool(name="tmpp", bufs=1))
    sbg2p = ctx.enter_context(tc.tile_pool(name="sbg2p", bufs=4))
    small = ctx.enter_context(tc.tile_pool(name="small", bufs=3))

    # persistent SBUF state
    u1 = per.tile([128, NBG * K * CO], BF16)  # resident u
    u1v = u1[:].rearrange("p (k b x) -> p k b x", k=K, b=NBG)
    wt_sb = per.tile([128, K * CO], BF16)
    xp_sb = per.tile([128, K * BL], BF16)
    logits = per.tile([128, NBG * K * C], F32)
    logv = logits[:].rearrange("p (b k c) -> p b k c", b=NBG, k=K)
    cexp = per.tile([128, NBG * K * C], BF16)
    cexpv = cexp[:].rearrange("p (b k c) -> p b k c", b=NBG, k=K)
    sel_t = per.tile([8, 128], BF16)
    msk_t = per.tile([128, 8], BF16)
    mskf_t = per.tile([128, 8], F32)
    vb_a = per.tile([128, NBG * CO], BF16, tag="vb_a")
    vb_b = per.tile([128, NBG * CO], BF16, tag="vb_b")
    vb_ab = [vb_a, vb_b]
    # Block-diag c staging for the s-pass: up to two units are in
    # flight between emit_compute and emit_spass, so 2 units x 2 halves.
    KH = K // 2
    cbd_ab = [
        per.tile([128, KH * C * 8], BF16, tag=f"cbd{h}", name=f"cbd{h}")
        for h in range(4)
    ]
    cbd_slot = {}
    cbd_ctr = [0]

    nc.sync.dma_start(out=sel_t[:], in_=sel_d)
    nc.sync.dma_start(out=msk_t[:], in_=msk_d)
    nc.gpsimd.dma_start(out=xp_sb[:], in_=xp_d)
    nc.scalar.copy(mskf_t[:], msk_t[:])
    # weights: write-once resident buffer, grouped DMAs on the gp queue.
    # Leading small groups shorten the ramp for bg0's first matmuls.
    if K % 8 == 0:
        wgroups = [2, 2, 4] + [8] * ((K - 8) // 8)
    else:
        wgroups = [1] * K
    k0 = 0
    for KB in wgroups:
        nc.gpsimd.dma_start(
            out=wt_sb[:, k0 * CO : (k0 + KB) * CO],
            in_=wt_d[:, k0 * CO : (k0 + KB) * CO],
        )
        k0 += KB

    def vb_broadcast(bg, v_bg_ap, vb_dst):
        """broadcast v (partitions 0..7) into all 128 partitions of vb_dst."""
        vbf_bg = small.tile([8, CO], BF16, tag="vbf_bg")
        nc.scalar.copy(vbf_bg[:], v_bg_ap)
        vbp = vbpsum.tile([128, CO], F32, tag="vbp")
        nc.tensor.matmul(vbp[:], lhsT=sel_t[:], rhs=vbf_bg[:], start=True, stop=True)
        nc.scalar.copy(vb_dst[:, bg * CO : (bg + 1) * CO], vbp[:])

    # ---------------- wavefront emission ----------------
    # Per-bg chains (u -> it1 -> it2 -> it3 -> out) are independent, so
    # emission interleaves them: u(0) it1(0) u(1) it2(0) it1(1) u(2)
    # it2(1) it1(2) u(3) it2(2) it1(3) it2(3) it3(0..3).  Each engine's
    # in-order queue then matches execution time: the DVE runs the delta
    # chain back-to-back while the tensor engine computes later groups'
    # u and earlier groups' s-pass.
    s1_bg = per.tile([8, NBG * CO], F32)  # s1 rebased to partitions 0..7
    s_sb = per.tile([BL, CO], F32)
    if K % 8 == 0:
        groups0 = [2, 2, 4] + [8] * ((K - 8) // 8)
    else:
        groups0 = [1] * K
    if K % 12 == 0:
        groupsN = [12] * (K // 12)
    else:
        groupsN = groups0
    KBMAX = max(max(groups0), max(groupsN))
    KC = 3 if K % 3 == 0 else 1  # u chunks per PSUM bank / scalar copy

    def emit_u(bg, mid_hook=None, s1ps=None):
        groups = groups0 if bg == 0 else groupsN
        k0 = 0
        ups = None
        for KB in groups:
            if mid_hook is not None and k0 >= K // 4:
                mid_hook()
                mid_hook = None
            xbd_t = xbdp.tile([128, KBMAX * 128], BF16, tag="xbd")
            nc.sync.dma_start(
                out=xbd_t[:, : KB * 128],
                in_=xbd_d[bg, :, k0 * 128 : (k0 + KB) * 128],
            )
            for kk in range(KB):
                k = k0 + kk
                if k % KC == 0:
                    ups = upsum.tile([128, KC * CO], F32, tag="ups")
                nc.tensor.matmul(
                    ups[:, (k % KC) * CO : (k % KC + 1) * CO],
                    lhsT=xbd_t[:, kk * 128 : (kk + 1) * 128],
                    rhs=wt_sb[:, k * CO : (k + 1) * CO],
                    start=True,
                    stop=True,
                )
                if s1ps is not None:
                    # iter-1 shortcut for ALL batches: x for every group
                    # is resident (xp_sb), so the full s1 accumulates
                    # during bg0's pass alone and it1 for every group can
                    # start the moment u(0) finishes -- later groups'
                    # deltas then wait only on their own u, not on a
                    # squash chain hanging off their u-phase tail.
                    nc.tensor.matmul(
                        s1ps[:],
                        lhsT=xp_sb[:, k * BL : (k + 1) * BL],
                        rhs=wt_sb[:, k * CO : (k + 1) * CO],
                        start=(k == 0),
                        stop=(k == K - 1),
                    )
                if k % KC == KC - 1:
                    nc.scalar.copy(
                        u1v[:, k - KC + 1 : k + 1, bg],
                        ups[:].rearrange("p (h x) -> p h x", h=KC),
                    )
            k0 += KB

    def emit_it1_all(s1ps):
        # evacuate the full s1, rebase each group to partitions 0..7,
        # then squash + broadcast all four groups back-to-back (their
        # DVE ops sit at the head of the queue, before any delta).
        nc.scalar.copy(s_sb[:], s1ps[:])
        for bg in range(NBG):
            nc.sync.dma_start(
                out=s1_bg[:, bg * CO : (bg + 1) * CO],
                in_=s_sb[bg * 8 : (bg + 1) * 8, :],
            )
        for bg in range(NBG):
            v_bg = small.tile([8, CO], F32, tag="v_bg")
            _squash_sg(
                nc, small, s1_bg[:, bg * CO : (bg + 1) * CO], v_bg[:],
                pre=1.0 / C,
            )
            vb_broadcast(bg, v_bg[:], vb_ab[0])

    def emit_compute(it, bg):
        """DVE delta chain + scalar softmax + gp cbd build for one unit.
        No PE ops: the s-pass is emitted separately (emit_spass) so the
        tensor-engine queue keeps later u-phases ahead of s-passes."""
        vb = vb_ab[it % 2]
        # delta for both halves first: keeps the DVE busy on mult/reduce
        # while the scalar queue catches up to the exps.
        for kh in range(2):
            ks = kh * KH
            # delta[b,r,c] = sum_o u*v (vector engine; mul at 2x,
            # reduce at its 1x ISA cap -- the kernel's pacing chain)
            tmpt = tmpp.tile([128, KH * CO], BF16, tag="tmp")
            nc.vector.tensor_tensor(
                out=tmpt[:].rearrange("p (k x) -> p k x", k=KH),
                in0=u1v[:, ks : ks + KH, bg],
                in1=vb[:, bg * CO : (bg + 1) * CO]
                .unsqueeze(1)
                .broadcast_to([128, KH, CO]),
                op=ALU.mult,
            )
            red_in = tmpt[:].rearrange("p (k c o) -> p k c o", k=KH, c=C)
            lh = logv[:, bg, ks : ks + KH]
            if it == 2:
                nc.vector.reduce_sum(out=lh, in_=red_in, axis=AX.X)
            else:
                dtm = small.tile([128, KH * C], F32, tag="dtm")
                nc.vector.reduce_sum(
                    out=dtm[:].rearrange("p (k c) -> p k c", k=KH),
                    in_=red_in,
                    axis=AX.X,
                )
                nc.vector.tensor_tensor(
                    out=lh.rearrange("p k c -> p (k c)"),
                    in0=lh.rearrange("p k c -> p (k c)"),
                    in1=dtm[:],
                    op=ALU.add,
                )
            # softmax exp early on the scalar queue
            ch = cexpv[:, bg, ks : ks + KH]
            nc.scalar.activation(ch, lh, AF.Exp)
        for kh in range(2):
            ks = kh * KH
            ch = cexpv[:, bg, ks : ks + KH]
            sume = small.tile([128, KH], F32, tag="sume")
            nc.vector.reduce_sum(out=sume[:], in_=ch, axis=AX.X)
            rs = small.tile([128, KH], F32, tag="rs")
            nc.vector.reciprocal(rs[:], sume[:])
            rsb = small.tile([128, KH], BF16, tag="rsb")
            nc.scalar.copy(rsb[:], rs[:])
            nc.vector.tensor_tensor(
                out=ch,
                in0=ch,
                in1=rsb[:].unsqueeze(2).broadcast_to([128, KH, C]),
                op=ALU.mult,
            )
            # stage block-diag c for the s-pass: 8 scalar-engine copies
            # (one per diag column e), each scaled by the per-partition
            # mask column so only partitions with p//16 == e land
            # nonzero.  This keeps the build off gpsimd entirely: a
            # gpsimd op in flight stalls concurrent small DVE ops, and
            # the 5us gpsimd build also sat on the s-pass critical path.
            if kh == 0:
                cbd_slot[(it, bg)] = cbd_ctr[0] % 2
                cbd_ctr[0] += 1
            cbd_t = cbd_ab[2 * cbd_slot[(it, bg)] + kh]
            cbdv = cbd_t[:].rearrange("p (k c e) -> p k c e", k=KH, c=C)
            for e in range(8):
                nc.scalar.activation(
                    cbdv[:, :, :, e], ch, AF.Identity,
                    scale=mskf_t[:, e : e + 1],
                )

    def emit_spass(it, bg, defer):
        """PE s-pass + diagonal extract for one unit; the squash is
        appended to `defer` for the caller to place in the queues."""
        vb_next = vb_ab[(it + 1) % 2]
        sps = spsum.tile([80, CO], F32, tag="sps")
        for kh in range(2):
            ks = kh * KH
            cbdv = cbd_ab[2 * cbd_slot.pop((it, bg)) + kh][:].rearrange(
                "p (k c e) -> p k c e", k=KH, c=C
            ) if kh == 1 else cbd_ab[2 * cbd_slot[(it, bg)] + kh][:].rearrange(
                "p (k c e) -> p k c e", k=KH, c=C
            )
            for kk in range(KH):
                nc.tensor.matmul(
                    sps[:],
                    lhsT=cbdv[:, kk].rearrange("p c e -> p (c e)"),
                    rhs=u1v[:, ks + kk, bg],
                    start=(ks + kk == 0),
                    stop=(ks + kk == K - 1),
                )
        stmp = small.tile([80, CO], F32, tag="stmp")
        nc.scalar.copy(stmp[:], sps[:])
        # diagonal extract (c==c') via DMA, one [8,16] block per c
        s_bg2 = sbg2p.tile([8, CO], F32, tag="s_bg2")
        qs = (nc.sync, nc.gpsimd, nc.scalar)
        for c in range(C):
            qs[c % 3].dma_start(
                out=s_bg2[:, c * OC : (c + 1) * OC],
                in_=stmp[c * 8 : (c + 1) * 8, c * OC : (c + 1) * OC],
            )
        def deferred(bg=bg, s=s_bg2, it=it, vn=vb_next):
            v_bg2 = small.tile([8, CO], F32, tag="v_bg2")
            _squash_sg(nc, small, s[:], v_bg2[:], pre=1.0)
            if it == 2:
                vb_broadcast(bg, v_bg2[:], vn)
            else:
                nc.sync.dma_start(
                    out=out_d[bg * 8 : (bg + 1) * 8, :], in_=v_bg2[:]
                )

        defer.append(deferred)

    # The wavefront.  PE queue: u(0) vb(0) u(1) vb(1) u(2) vb(2) u(3)
    # vb(3) spass2(0..3) spass3(0..3) -- u-phases stay ahead of all
    # s-passes so the per-bg routing cycle is paced by the DVE alone.
    # compute(it,bg) is emitted inside u(bg+1)'s chunk loop (mid_hook) so
    # its exps interleave with the u-copies on the scalar queue at the
    # position matching their execution time.
    s1ps = s1psum.tile([BL, CO], F32, tag="s1ps")
    emit_u(0, s1ps=s1ps)
    emit_it1_all(s1ps)
    emit_u(1, mid_hook=lambda: emit_compute(2, 0))
    emit_u(2, mid_hook=lambda: emit_compute(2, 1))
    # it2 s-passes run on the PE as soon as each unit's c lands (the PE
    # is free once the u-phases finish); their squashes are spliced
    # between later delta units on the DVE so that by the time D2(3)
    # retires, vb_b for the early groups already exists and iteration 3
    # starts immediately.
    sq2 = []
    emit_spass(2, 0, sq2)
    emit_u(3, mid_hook=lambda: emit_compute(2, 2))
    sq2[0]()
    emit_spass(2, 1, sq2)
    emit_compute(2, 3)
    emit_spass(2, 2, sq2)
    sq2[1]()
    sq2[2]()
    emit_spass(2, 3, sq2)
    emit_compute(3, 0)
    sq2[3]()
    sq3 = []
    emit_spass(3, 0, sq3)
    for bg in range(1, NBG):
        emit_compute(3, bg)
        sq3[bg - 1]()
        emit_spass(3, bg, sq3)
    sq3[NBG - 1]()


def build(r=None):
    """Build and compile the Bass program. Returns the compiled Bacc."""
    K = _nchunks(r)
    nc = bacc.Bacc(
        "TRN2", target_bir_lowering=False, debug=False, num_devices=NCORES
    )
    xbd_d = nc.dram_tensor(
        "xbd", [NBG, 128, K * 128], BF16, kind="ExternalInput"
    ).ap()
    wt_d = nc.dram_tensor("wt", [128, K * CO], BF16, kind="ExternalInput").ap()
    xp_d = nc.dram_tensor("xp", [128, K * BL], BF16, kind="ExternalInput").ap()
    sel_d = nc.dram_tensor("sel", [8, 128], BF16, kind="ExternalInput").ap()
    msk_d = nc.dram_tensor("msk", [128, 8], BF16, kind="ExternalInput").ap()
    out_d = nc.dram_tensor("v_out", [BL, CO], F32, kind="ExternalOutput").ap()
    with tile.TileContext(nc) as tc, ExitStack() as ctx:
        _body(ctx, tc, xbd_d, wt_d, xp_d, sel_d, msk_d, out_d, K)
    nc.compile()
    return nc


def make_inputs(x, weights, r=None):
    """Host-side marshalling: shard x over cores, rearrange to bf16 tiles."""
    K = _nchunks(r)
    r_full = K * G
    W = np.asarray(weights, dtype=np.float32)[0][:r_full]  # [R, C, IC, OC]
    wt = (
        W.reshape(K, G, C, IC, OC)
        .transpose(0, 1, 3, 2, 4)
        .reshape(K, 128, CO)
        .transpose(1, 0, 2)
        .reshape(128, K * CO)
        .astype(NPBF)
    )
    sel = np.zeros((8, 128), dtype=np.float32)
    bi = np.arange(8)
    gi = np.arange(G)
    sel[bi[:, None], bi[:, None] * G + gi[None, :]] = 1.0
    sel = sel.astype(NPBF)
    msk = np.zeros((128, 8), dtype=np.float32)
    pi = np.arange(128)
    msk[pi, pi // G] = 1.0
    msk = msk.astype(NPBF)

    in_maps = []
    xf = np.asarray(x, dtype=np.float32)[:, :r_full]
    for core in range(NCORES):
        xl = xf[core * BL : (core + 1) * BL]  # [BL, R, IC]
        xr = xl.transpose(1, 2, 0).reshape(K, G, IC, BL)  # [K, g, i, b]
        xp = (
            xr.reshape(K, 128, BL).transpose(1, 0, 2).reshape(128, K * BL)
        ).astype(NPBF)
        xrg = xr.reshape(K, G, IC, NBG, 8)
        xbd6 = np.zeros((K, G, IC, NBG, 8, G), dtype=np.float32)
        for g in range(G):
            xbd6[:, g, :, :, :, g] = xrg[:, g]
        xbd = (
            xbd6.reshape(K, 128, NBG, 128)
            .transpose(2, 1, 0, 3)
            .reshape(NBG, 128, K * 128)
            .astype(NPBF)
        )
        in_maps.append(
            {"xbd": xbd, "wt": wt, "xp": xp, "sel": sel, "msk": msk}
        )
    return in_maps


_CACHE = {}


def kernel(x, weights):
    if "nc" not in _CACHE:
        _CACHE["nc"] = build()
    nc = _CACHE["nc"]
    in_maps = make_inputs(x, weights)
    res = run_bass_kernel_spmd(nc, in_maps, core_ids=list(range(NCORES)))
    outs = [res.results[i]["v_out"].reshape(BL, C, OC) for i in range(NCORES)]
    return np.concatenate(outs, axis=0)


# revision 39
# speedup vs baseline: 1.2639x; 1.2639x over previous
"""DigitCaps dynamic-routing kernel for Trainium2 (8 NeuronCores, SPMD).

Problem:  u = einsum('bri,rcio->brco', x, W[0]);  3 routing iterations
          (softmax over capsules, weighted sum over routes, squash,
          agreement update);  returns v [B, C, OC].

Shapes: B=256, R=1152, C=10, IC=8, OC=16.  Batch-sharded 8 ways (BL=32
per core, zero cross-core communication).

Design notes (per core) -- batch-group-major software pipeline:
 - The 32 batches are processed as 4 groups of 8 (bg).  Routing is
   per-batch independent, so each bg's chain (u-phase -> it1 -> it2 ->
   it3 -> output) only depends on its own u.  The u-phase for bg1..3
   (tensor engine) therefore overlaps the routing iterations of earlier
   groups (vector engine), hiding most of the u-phase behind the DVE
   critical path instead of serializing phase-by-phase.
 - u-phase per bg: one matmul per 16-route chunk; lhsT is block-diag x
   ([128=(g,i), 128=(b8,g16)]), rhs the W chunk [128=(g,i), 160=(c,o)],
   fp32 PSUM, evacuated to resident bf16 u1 by the scalar engine only
   (the DVE never touches u-phase data: it is the kernel bottleneck).
 - Iter-1 shortcut: c is uniform 1/10, so s1 = 0.1*sum_r u via one
   extra accumulating matmul per chunk (plain x as lhsT, 8 rows per bg).
 - b-update (delta = sum_o u*v) is DVE mult (bf16 2x) + reduce (1x
   cap); softmax sum/reciprocal on DVE, exp on scalar.  These are the
   only DVE ops in steady state, so the pipeline is paced by them.
 - squash runs on scalar+gpsimd only (gp divide for the normalize), so
   per-bg squashes never head-of-line-block the DVE queue.
 - s-pass (s = sum_r c*u) on the tensor engine: lhsT is block-diag c
   built by a gpsimd mask-multiply, PSUM-accumulated over all 72
   chunks; the (c,c') diagonal is extracted with small DMAs.
 - Outputs stream out per bg (4 small DMAs) so the tail is short.
"""

import sys

sys.path.insert(0, "/opt/trn_rl_repo")

from contextlib import ExitStack

import ml_dtypes
import numpy as np

import concourse.bass as bass
import concourse.tile as tile
from concourse import bacc, mybir
from concourse.bass_utils import run_bass_kernel_spmd

BF16 = mybir.dt.bfloat16
F32 = mybir.dt.float32
AF = mybir.ActivationFunctionType
ALU = mybir.AluOpType
AX = mybir.AxisListType

B, R, C, IC, OC = 256, 1152, 10, 8, 16
NCORES = 8
BL = B // NCORES  # 32 batches per core
G = 16  # routes per chunk
NBG = BL // 8  # 4 b-groups of 8
CO = C * OC  # 160
EPS = 1e-8
NPBF = ml_dtypes.bfloat16

# Set by tests to shrink the problem for simulation; full size by default.
_R_OVERRIDE = None


def _nchunks(r=None):
    r = r if r is not None else (_R_OVERRIDE or R)
    assert r % G == 0
    return r // G


def _squash_sg(nc, pool, s_ap, v_ap, pre, np_=8):
    """v = squash(pre*s), DVE-centric: the ops are tiny ([8,C]-ish) and
    call sites defer emission by one bg-unit so they slot into the DVE
    queue right after the following group's delta chain instead of
    blocking it.  Only the sqrt leaves the DVE (one scalar-engine hop) --
    cross-engine ping-pong latency, not op cost, is what hurt here.

    With q = sum_o s^2 (unscaled), the scale factor is
      sc = pre^3*q / ((pre*sqrt(q) + EPS) * (1 + pre^2*q)),   v = sc * s.

    Uses sqrt(n2 + EPS^2) ~= nrm + EPS (difference is O(EPS) absolute,
    only relevant when the norm itself is ~EPS).
    """
    sq = pool.tile([np_, CO], F32, tag="sq")
    if s_ap.space == bass.MemorySpace.PSUM:
        # a PSUM tensor may be read only once per instruction
        nc.scalar.square(sq[:], s_ap)
    else:
        nc.vector.tensor_tensor(out=sq[:], in0=s_ap, in1=s_ap, op=ALU.mult)
    q = pool.tile([np_, C], F32, tag="n2")
    nc.vector.reduce_sum(
        out=q[:], in_=sq[:].rearrange("p (c o) -> p c o", c=C), axis=AX.X
    )
    nrm = pool.tile([np_, C], F32, tag="nrm")
    nc.scalar.sqrt(nrm[:], q[:])
    t1 = pool.tile([np_, C], F32, tag="t1")
    nc.vector.tensor_scalar(
        out=t1[:], in0=q[:], scalar1=pre * pre, scalar2=1.0,
        op0=ALU.mult, op1=ALU.add,
    )
    den = pool.tile([np_, C], F32, tag="den")
    nc.vector.tensor_scalar(
        out=den[:], in0=nrm[:], scalar1=pre, scalar2=EPS,
        op0=ALU.mult, op1=ALU.add,
    )
    nc.vector.tensor_tensor(out=den[:], in0=den[:], in1=t1[:], op=ALU.mult)
    rden = pool.tile([np_, C], F32, tag="rden")
    nc.vector.reciprocal(rden[:], den[:])
    sc = pool.tile([np_, C], F32, tag="sc")
    nc.vector.scalar_tensor_tensor(
        out=sc[:], in0=q[:], scalar=pre * pre * pre, in1=rden[:],
        op0=ALU.mult, op1=ALU.mult,
    )
    nc.vector.tensor_tensor(
        out=v_ap.rearrange("p (c o) -> p c o", c=C),
        in0=s_ap.rearrange("p (c o) -> p c o", c=C),
        in1=sc[:].unsqueeze(2).broadcast_to([np_, C, OC]),
        op=ALU.mult,
    )


def _body(ctx, tc, xbd_d, wt_d, xp_d, sel_d, msk_d, out_d, K):
    nc = tc.nc

    per = ctx.enter_context(tc.tile_pool(name="per", bufs=1))
    xbdp = ctx.enter_context(tc.tile_pool(name="xbdp", bufs=4))
    upsum = ctx.enter_context(tc.tile_pool(name="upsum", bufs=3, space="PSUM"))
    s1psum = ctx.enter_context(tc.tile_pool(name="s1psum", bufs=1, space="PSUM"))
    spsum = ctx.enter_context(tc.tile_pool(name="spsum", bufs=2, space="PSUM"))
    vbpsum = ctx.enter_context(tc.tile_pool(name="vbpsum", bufs=2, space="PSUM"))
    tmpp = ctx.enter_context(tc.tile_pool(name="tmpp", bufs=1))
    sbg2p = ctx.enter_context(tc.tile_pool(name="sbg2p", bufs=4))
    small = ctx.enter_context(tc.tile_pool(name="small", bufs=3))

    # persistent SBUF state
    u1 = per.tile([128, NBG * K * CO], BF16)  # resident u
    u1v = u1[:].rearrange("p (k b x) -> p k b x", k=K, b=NBG)
    wt_sb = per.tile([128, K * CO], BF16)
    xp_sb = per.tile([128, K * BL], BF16)
    logits = per.tile([128, NBG * K * C], F32)
    logv = logits[:].rearrange("p (b k c) -> p b k c", b=NBG, k=K)
    cexp = per.tile([128, NBG * K * C], BF16)
    cexpv = cexp[:].rearrange("p (b k c) -> p b k c", b=NBG, k=K)
    sel_t = per.tile([8, 128], BF16)
    msk_t = per.tile([128, 8], BF16)
    mskf_t = per.tile([128, 8], F32)
    vb_a = per.tile([128, NBG * CO], BF16, tag="vb_a")
    vb_b = per.tile([128, NBG * CO], BF16, tag="vb_b")
    vb_ab = [vb_a, vb_b]
    # Block-diag c staging for the s-pass: up to two units are in
    # flight between emit_compute and emit_spass, so 2 units x 2 halves.
    KH = K // 2
    cbd_ab = [
        per.tile([128, KH * C * 8], BF16, tag=f"cbd{h}", name=f"cbd{h}")
        for h in range(4)
    ]
    cbd_slot = {}
    cbd_ctr = [0]

    nc.sync.dma_start(out=sel_t[:], in_=sel_d)
    nc.sync.dma_start(out=msk_t[:], in_=msk_d)
    nc.gpsimd.dma_start(out=xp_sb[:], in_=xp_d)
    nc.scalar.copy(mskf_t[:], msk_t[:])
    # weights: write-once resident buffer, grouped DMAs on the gp queue.
    # Leading small groups shorten the ramp for bg0's first matmuls.
    if K % 8 == 0:
        wgroups = [2, 2, 4] + [8] * ((K - 8) // 8)
    else:
        wgroups = [1] * K
    k0 = 0
    for KB in wgroups:
        nc.gpsimd.dma_start(
            out=wt_sb[:, k0 * CO : (k0 + KB) * CO],
            in_=wt_d[:, k0 * CO : (k0 + KB) * CO],
        )
        k0 += KB

    def vb_broadcast(bg, v_bg_ap, vb_dst):
        """broadcast v (partitions 0..7) into all 128 partitions of vb_dst."""
        vbf_bg = small.tile([8, CO], BF16, tag="vbf_bg")
        nc.scalar.copy(vbf_bg[:], v_bg_ap)
        vbp = vbpsum.tile([128, CO], F32, tag="vbp")
        nc.tensor.matmul(vbp[:], lhsT=sel_t[:], rhs=vbf_bg[:], start=True, stop=True)
        nc.scalar.copy(vb_dst[:, bg * CO : (bg + 1) * CO], vbp[:])

    # ---------------- wavefront emission ----------------
    # Per-bg chains (u -> it1 -> it2 -> it3 -> out) are independent, so
    # emission interleaves them: u(0) it1(0) u(1) it2(0) it1(1) u(2)
    # it2(1) it1(2) u(3) it2(2) it1(3) it2(3) it3(0..3).  Each engine's
    # in-order queue then matches execution time: the DVE runs the delta
    # chain back-to-back while the tensor engine computes later groups'
    # u and earlier groups' s-pass.
    s1_bg = per.tile([8, NBG * CO], F32)  # s1 rebased to partitions 0..7
    s_sb = per.tile([BL, CO], F32)
    if K % 8 == 0:
        groups0 = [2, 2, 4] + [8] * ((K - 8) // 8)
    else:
        groups0 = [1] * K
    if K % 12 == 0:
        groupsN = [12] * (K // 12)
    else:
        groupsN = groups0
    KBMAX = max(max(groups0), max(groupsN))
    KC = 3 if K % 3 == 0 else 1  # u chunks per PSUM bank / scalar copy

    def emit_u(bg, mid_hook=None, s1ps=None):
        groups = groups0 if bg == 0 else groupsN
        k0 = 0
        ups = None
        for KB in groups:
            if mid_hook is not None and k0 >= K // 4:
                mid_hook()
                mid_hook = None
            xbd_t = xbdp.tile([128, KBMAX * 128], BF16, tag="xbd")
            nc.sync.dma_start(
                out=xbd_t[:, : KB * 128],
                in_=xbd_d[bg, :, k0 * 128 : (k0 + KB) * 128],
            )
            for kk in range(KB):
                k = k0 + kk
                if k % KC == 0:
                    ups = upsum.tile([128, KC * CO], F32, tag="ups")
                nc.tensor.matmul(
                    ups[:, (k % KC) * CO : (k % KC + 1) * CO],
                    lhsT=xbd_t[:, kk * 128 : (kk + 1) * 128],
                    rhs=wt_sb[:, k * CO : (k + 1) * CO],
                    start=True,
                    stop=True,
                )
                if s1ps is not None:
                    # iter-1 shortcut for ALL batches: x for every group
                    # is resident (xp_sb), so the full s1 accumulates
                    # during bg0's pass alone and it1 for every group can
                    # start the moment u(0) finishes -- later groups'
                    # deltas then wait only on their own u, not on a
                    # squash chain hanging off their u-phase tail.
                    nc.tensor.matmul(
                        s1ps[:],
                        lhsT=xp_sb[:, k * BL : (k + 1) * BL],
                        rhs=wt_sb[:, k * CO : (k + 1) * CO],
                        start=(k == 0),
                        stop=(k == K - 1),
                    )
                if k % KC == KC - 1:
                    nc.scalar.copy(
                        u1v[:, k - KC + 1 : k + 1, bg],
                        ups[:].rearrange("p (h x) -> p h x", h=KC),
                    )
            k0 += KB

    def emit_it1_all(s1ps):
        # evacuate the full s1, rebase each group to partitions 0..7,
        # then squash + broadcast all four groups back-to-back (their
        # DVE ops sit at the head of the queue, before any delta).
        nc.scalar.copy(s_sb[:], s1ps[:])
        for bg in range(NBG):
            nc.sync.dma_start(
                out=s1_bg[:, bg * CO : (bg + 1) * CO],
                in_=s_sb[bg * 8 : (bg + 1) * 8, :],
            )
        for bg in range(NBG):
            v_bg = small.tile([8, CO], F32, tag="v_bg")
            _squash_sg(
                nc, small, s1_bg[:, bg * CO : (bg + 1) * CO], v_bg[:],
                pre=1.0 / C,
            )
            vb_broadcast(bg, v_bg[:], vb_ab[0])

    def emit_compute(it, bg):
        """DVE delta chain + scalar softmax + gp cbd build for one unit.
        No PE ops: the s-pass is emitted separately (emit_spass) so the
        tensor-engine queue keeps later u-phases ahead of s-passes."""
        vb = vb_ab[it % 2]
        # delta for both halves first: keeps the DVE busy on mult/reduce
        # while the scalar queue catches up to the exps.
        for kh in range(2):
            ks = kh * KH
            # delta[b,r,c] = sum_o u*v (vector engine; mul at 2x,
            # reduce at its 1x ISA cap -- the kernel's pacing chain)
            tmpt = tmpp.tile([128, KH * CO], BF16, tag="tmp")
            nc.vector.tensor_tensor(
                out=tmpt[:].rearrange("p (k x) -> p k x", k=KH),
                in0=u1v[:, ks : ks + KH, bg],
                in1=vb[:, bg * CO : (bg + 1) * CO]
                .unsqueeze(1)
                .broadcast_to([128, KH, CO]),
                op=ALU.mult,
            )
            red_in = tmpt[:].rearrange("p (k c o) -> p k c o", k=KH, c=C)
            lh = logv[:, bg, ks : ks + KH]
            if it == 2:
                nc.vector.reduce_sum(out=lh, in_=red_in, axis=AX.X)
            else:
                dtm = small.tile([128, KH * C], F32, tag="dtm")
                nc.vector.reduce_sum(
                    out=dtm[:].rearrange("p (k c) -> p k c", k=KH),
                    in_=red_in,
                    axis=AX.X,
                )
                nc.vector.tensor_tensor(
                    out=lh.rearrange("p k c -> p (k c)"),
                    in0=lh.rearrange("p k c -> p (k c)"),
                    in1=dtm[:],
                    op=ALU.add,
                )
            # softmax exp early on the scalar queue
            ch = cexpv[:, bg, ks : ks + KH]
            nc.scalar.activation(ch, lh, AF.Exp)
        for kh in range(2):
            ks = kh * KH
            ch = cexpv[:, bg, ks : ks + KH]
            sume = small.tile([128, KH], F32, tag="sume")
            nc.vector.reduce_sum(out=sume[:], in_=ch, axis=AX.X)
            rs = small.tile([128, KH], F32, tag="rs")
            nc.vector.reciprocal(rs[:], sume[:])
            rsb = small.tile([128, KH], BF16, tag="rsb")
            nc.scalar.copy(rsb[:], rs[:])
            nc.vector.tensor_tensor(
                out=ch,
                in0=ch,
                in1=rsb[:].unsqueeze(2).broadcast_to([128, KH, C]),
                op=ALU.mult,
            )
            # build block-diag c = c (x) delta-mask on gpsimd
            if kh == 0:
                cbd_slot[(it, bg)] = cbd_ctr[0] % 2
                cbd_ctr[0] += 1
            cbd_t = cbd_ab[2 * cbd_slot[(it, bg)] + kh]
            cbdv = cbd_t[:].rearrange("p (k c e) -> p k c e", k=KH, c=C)
            nc.gpsimd.tensor_tensor(
                out=cbdv,
                in0=ch.unsqueeze(3).broadcast_to([128, KH, C, 8]),
                in1=msk_t[:]
                .unsqueeze(1)
                .unsqueeze(1)
                .broadcast_to([128, KH, C, 8]),
                op=ALU.mult,
            )

    def emit_spass(it, bg, defer):
        """PE s-pass + diagonal extract for one unit; the squash is
        appended to `defer` for the caller to place in the queues."""
        vb_next = vb_ab[(it + 1) % 2]
        sps = spsum.tile([80, CO], F32, tag="sps")
        for kh in range(2):
            ks = kh * KH
            cbdv = cbd_ab[2 * cbd_slot.pop((it, bg)) + kh][:].rearrange(
                "p (k c e) -> p k c e", k=KH, c=C
            ) if kh == 1 else cbd_ab[2 * cbd_slot[(it, bg)] + kh][:].rearrange(
                "p (k c e) -> p k c e", k=KH, c=C
            )
            for kk in range(KH):
                nc.tensor.matmul(
                    sps[:],
                    lhsT=cbdv[:, kk].rearrange("p c e -> p (c e)"),
                    rhs=u1v[:, ks + kk, bg],
                    start=(ks + kk == 0),
                    stop=(ks + kk == K - 1),
                )
        stmp = small.tile([80, CO], F32, tag="stmp")
        nc.scalar.copy(stmp[:], sps[:])
        # diagonal extract (c==c') via DMA, one [8,16] block per c
        s_bg2 = sbg2p.tile([8, CO], F32, tag="s_bg2")
        qs = (nc.sync, nc.gpsimd, nc.scalar)
        for c in range(C):
            qs[c % 3].dma_start(
                out=s_bg2[:, c * OC : (c + 1) * OC],
                in_=stmp[c * 8 : (c + 1) * 8, c * OC : (c + 1) * OC],
            )
        def deferred(bg=bg, s=s_bg2, it=it, vn=vb_next):
            v_bg2 = small.tile([8, CO], F32, tag="v_bg2")
            _squash_sg(nc, small, s[:], v_bg2[:], pre=1.0)
            if it == 2:
                vb_broadcast(bg, v_bg2[:], vn)
            else:
                nc.sync.dma_start(
                    out=out_d[bg * 8 : (bg + 1) * 8, :], in_=v_bg2[:]
                )

        defer.append(deferred)

    # The wavefront.  PE queue: u(0) vb(0) u(1) vb(1) u(2) vb(2) u(3)
    # vb(3) spass2(0..3) spass3(0..3) -- u-phases stay ahead of all
    # s-passes so the per-bg routing cycle is paced by the DVE alone.
    # compute(it,bg) is emitted inside u(bg+1)'s chunk loop (mid_hook) so
    # its exps interleave with the u-copies on the scalar queue at the
    # position matching their execution time.
    s1ps = s1psum.tile([BL, CO], F32, tag="s1ps")
    emit_u(0, s1ps=s1ps)
    emit_it1_all(s1ps)
    emit_u(1, mid_hook=lambda: emit_compute(2, 0))
    emit_u(2, mid_hook=lambda: emit_compute(2, 1))
    # it2 s-passes run on the PE as soon as each unit's c lands (the PE
    # is free once the u-phases finish); their squashes are spliced
    # between later delta units on the DVE so that by the time D2(3)
    # retires, vb_b for the early groups already exists and iteration 3
    # starts immediately.
    sq2 = []
    emit_spass(2, 0, sq2)
    emit_u(3, mid_hook=lambda: emit_compute(2, 2))
    sq2[0]()
    emit_spass(2, 1, sq2)
    emit_compute(2, 3)
    emit_spass(2, 2, sq2)
    sq2[1]()
    sq2[2]()
    emit_spass(2, 3, sq2)
    emit_compute(3, 0)
    sq2[3]()
    sq3 = []
    emit_spass(3, 0, sq3)
    for bg in range(1, NBG):
        emit_compute(3, bg)
        sq3[bg - 1]()
        emit_spass(3, bg, sq3)
    sq3[NBG - 1]()


def build(r=None):
    """Build and compile the Bass program. Returns the compiled Bacc."""
    K = _nchunks(r)
    nc = bacc.Bacc(
        "TRN2", target_bir_lowering=False, debug=False, num_devices=NCORES
    )
    xbd_d = nc.dram_tensor(
        "xbd", [NBG, 128, K * 128], BF16, kind="ExternalInput"
    ).ap()
    wt_d = nc.dram_tensor("wt", [128, K * CO], BF16, kind="ExternalInput").ap()
    xp_d = nc.dram_tensor("xp", [128, K * BL], BF16, kind="ExternalInput").ap()
    sel_d = nc.dram_tensor("sel", [8, 128], BF16, kind="ExternalInput").ap()
    msk_d = nc.dram_tensor("msk", [128, 8], BF16, kind="ExternalInput").ap()
    out_d = nc.dram_tensor("v_out", [BL, CO], F32, kind="ExternalOutput").ap()
    with tile.TileContext(nc) as tc, ExitStack() as ctx:
        _body(ctx, tc, xbd_d, wt_d, xp_d, sel_d, msk_d, out_d, K)
    nc.compile()
    return nc


def make_inputs(x, weights, r=None):
    """Host-side marshalling: shard x over cores, rearrange to bf16 tiles."""
    K = _nchunks(r)
    r_full = K * G
    W = np.asarray(weights, dtype=np.float32)[0][:r_full]  # [R, C, IC, OC]
    wt = (
        W.reshape(K, G, C, IC, OC)
        .transpose(0, 1, 3, 2, 4)
        .reshape(K, 128, CO)
        .transpose(1, 0, 2)
        .reshape(128, K * CO)
        .astype(NPBF)
    )
    sel = np.zeros((8, 128), dtype=np.float32)
    bi = np.arange(8)
    gi = np.arange(G)
    sel[bi[:, None], bi[:, None] * G + gi[None, :]] = 1.0
    sel = sel.astype(NPBF)
    msk = np.zeros((128, 8), dtype=np.float32)
    pi = np.arange(128)
    msk[pi, pi // G] = 1.0
    msk = msk.astype(NPBF)

    in_maps = []
    xf = np.asarray(x, dtype=np.float32)[:, :r_full]
    for core in range(NCORES):
        xl = xf[core * BL : (core + 1) * BL]  # [BL, R, IC]
        xr = xl.transpose(1, 2, 0).reshape(K, G, IC, BL)  # [K, g, i, b]
        xp = (
            xr.reshape(K, 128, BL).transpose(1, 0, 2).reshape(128, K * BL)
        ).astype(NPBF)
        xrg = xr.reshape(K, G, IC, NBG, 8)
        xbd6 = np.zeros((K, G, IC, NBG, 8, G), dtype=np.float32)
        for g in range(G):
            xbd6[:, g, :, :, :, g] = xrg[:, g]
        xbd = (
            xbd6.reshape(K, 128, NBG, 128)
            .transpose(2, 1, 0, 3)
            .reshape(NBG, 128, K * 128)
            .astype(NPBF)
        )
        in_maps.append(
            {"xbd": xbd, "wt": wt, "xp": xp, "sel": sel, "msk": msk}
        )
    return in_maps


_CACHE = {}


def kernel(x, weights):
    if "nc" not in _CACHE:
        _CACHE["nc"] = build()
    nc = _CACHE["nc"]
    in_maps = make_inputs(x, weights)
    res = run_bass_kernel_spmd(nc, in_maps, core_ids=list(range(NCORES)))
    outs = [res.results[i]["v_out"].reshape(BL, C, OC) for i in range(NCORES)]
    return np.concatenate(outs, axis=0)


# revision 40
# speedup vs baseline: 1.3223x; 1.0462x over previous
"""DigitCaps dynamic-routing kernel for Trainium2 (8 NeuronCores, SPMD).

Problem:  u = einsum('bri,rcio->brco', x, W[0]);  3 routing iterations
          (softmax over capsules, weighted sum over routes, squash,
          agreement update);  returns v [B, C, OC].

Shapes: B=256, R=1152, C=10, IC=8, OC=16.  Batch-sharded 8 ways (BL=32
per core, zero cross-core communication).

Design notes (per core) -- batch-group-major software pipeline:
 - The 32 batches are processed as 4 groups of 8 (bg).  Routing is
   per-batch independent, so each bg's chain (u-phase -> it1 -> it2 ->
   it3 -> output) only depends on its own u.  The u-phase for bg1..3
   (tensor engine) therefore overlaps the routing iterations of earlier
   groups (vector engine), hiding most of the u-phase behind the DVE
   critical path instead of serializing phase-by-phase.
 - u-phase per bg: one matmul per 16-route chunk; lhsT is block-diag x
   ([128=(g,i), 128=(b8,g16)]), rhs the W chunk [128=(g,i), 160=(c,o)],
   fp32 PSUM, evacuated to resident bf16 u1 by the scalar engine only
   (the DVE never touches u-phase data: it is the kernel bottleneck).
 - Iter-1 shortcut: c is uniform 1/10, so s1 = 0.1*sum_r u via one
   extra accumulating matmul per chunk (plain x as lhsT, 8 rows per bg).
 - b-update (delta = sum_o u*v) is DVE mult (bf16 2x) + reduce (1x
   cap); softmax sum/reciprocal on DVE, exp on scalar.  These are the
   only DVE ops in steady state, so the pipeline is paced by them.
 - squash runs on scalar+gpsimd only (gp divide for the normalize), so
   per-bg squashes never head-of-line-block the DVE queue.
 - s-pass (s = sum_r c*u) on the tensor engine: lhsT is block-diag c
   built by a gpsimd mask-multiply, PSUM-accumulated over all 72
   chunks; the (c,c') diagonal is extracted with small DMAs.
 - Outputs stream out per bg (4 small DMAs) so the tail is short.
"""

import sys

sys.path.insert(0, "/opt/trn_rl_repo")

from contextlib import ExitStack

import ml_dtypes
import numpy as np

import concourse.bass as bass
import concourse.tile as tile
from concourse import bacc, mybir
from concourse.bass_utils import run_bass_kernel_spmd

BF16 = mybir.dt.bfloat16
F32 = mybir.dt.float32
AF = mybir.ActivationFunctionType
ALU = mybir.AluOpType
AX = mybir.AxisListType

B, R, C, IC, OC = 256, 1152, 10, 8, 16
NCORES = 8
BL = B // NCORES  # 32 batches per core
G = 16  # routes per chunk
NBG = BL // 8  # 4 b-groups of 8
CO = C * OC  # 160
EPS = 1e-8
NPBF = ml_dtypes.bfloat16

# Set by tests to shrink the problem for simulation; full size by default.
_R_OVERRIDE = None


def _nchunks(r=None):
    r = r if r is not None else (_R_OVERRIDE or R)
    assert r % G == 0
    return r // G


def _squash_sg(nc, pool, s_ap, v_ap, pre, np_=8):
    """v = squash(pre*s), DVE-centric: the ops are tiny ([8,C]-ish) and
    call sites defer emission by one bg-unit so they slot into the DVE
    queue right after the following group's delta chain instead of
    blocking it.  Only the sqrt leaves the DVE (one scalar-engine hop) --
    cross-engine ping-pong latency, not op cost, is what hurt here.

    With q = sum_o s^2 (unscaled), the scale factor is
      sc = pre^3*q / ((pre*sqrt(q) + EPS) * (1 + pre^2*q)),   v = sc * s.

    Uses sqrt(n2 + EPS^2) ~= nrm + EPS (difference is O(EPS) absolute,
    only relevant when the norm itself is ~EPS).
    """
    sq = pool.tile([np_, CO], F32, tag="sq")
    if s_ap.space == bass.MemorySpace.PSUM:
        # a PSUM tensor may be read only once per instruction
        nc.scalar.square(sq[:], s_ap)
    else:
        nc.vector.tensor_tensor(out=sq[:], in0=s_ap, in1=s_ap, op=ALU.mult)
    q = pool.tile([np_, C], F32, tag="n2")
    nc.vector.reduce_sum(
        out=q[:], in_=sq[:].rearrange("p (c o) -> p c o", c=C), axis=AX.X
    )
    nrm = pool.tile([np_, C], F32, tag="nrm")
    nc.scalar.sqrt(nrm[:], q[:])
    t1 = pool.tile([np_, C], F32, tag="t1")
    nc.vector.tensor_scalar(
        out=t1[:], in0=q[:], scalar1=pre * pre, scalar2=1.0,
        op0=ALU.mult, op1=ALU.add,
    )
    den = pool.tile([np_, C], F32, tag="den")
    nc.vector.tensor_scalar(
        out=den[:], in0=nrm[:], scalar1=pre, scalar2=EPS,
        op0=ALU.mult, op1=ALU.add,
    )
    nc.vector.tensor_tensor(out=den[:], in0=den[:], in1=t1[:], op=ALU.mult)
    rden = pool.tile([np_, C], F32, tag="rden")
    nc.vector.reciprocal(rden[:], den[:])
    sc = pool.tile([np_, C], F32, tag="sc")
    nc.vector.scalar_tensor_tensor(
        out=sc[:], in0=q[:], scalar=pre * pre * pre, in1=rden[:],
        op0=ALU.mult, op1=ALU.mult,
    )
    nc.vector.tensor_tensor(
        out=v_ap.rearrange("p (c o) -> p c o", c=C),
        in0=s_ap.rearrange("p (c o) -> p c o", c=C),
        in1=sc[:].unsqueeze(2).broadcast_to([np_, C, OC]),
        op=ALU.mult,
    )


def _body(ctx, tc, xbd_d, wt_d, xp_d, sel_d, sel4_d, msk_d, out_d, K):
    nc = tc.nc

    per = ctx.enter_context(tc.tile_pool(name="per", bufs=1))
    xbdp = ctx.enter_context(tc.tile_pool(name="xbdp", bufs=4))
    upsum = ctx.enter_context(tc.tile_pool(name="upsum", bufs=3, space="PSUM"))
    s1psum = ctx.enter_context(tc.tile_pool(name="s1psum", bufs=1, space="PSUM"))
    spsum = ctx.enter_context(tc.tile_pool(name="spsum", bufs=2, space="PSUM"))
    vbpsum = ctx.enter_context(tc.tile_pool(name="vbpsum", bufs=2, space="PSUM"))
    tmpp = ctx.enter_context(tc.tile_pool(name="tmpp", bufs=1))
    sbg2p = ctx.enter_context(tc.tile_pool(name="sbg2p", bufs=4))
    small = ctx.enter_context(tc.tile_pool(name="small", bufs=3))

    # persistent SBUF state
    u1 = per.tile([128, NBG * K * CO], BF16)  # resident u
    u1v = u1[:].rearrange("p (k b x) -> p k b x", k=K, b=NBG)
    wt_sb = per.tile([128, K * CO], BF16)
    xp_sb = per.tile([128, K * BL], BF16)
    logits = per.tile([128, NBG * K * C], F32)
    logv = logits[:].rearrange("p (b k c) -> p b k c", b=NBG, k=K)
    cexp = per.tile([128, NBG * K * C], BF16)
    cexpv = cexp[:].rearrange("p (b k c) -> p b k c", b=NBG, k=K)
    sel_t = per.tile([8, 128], BF16)
    sel4_t = per.tile([BL, NBG * 128], BF16)
    msk_t = per.tile([128, 8], BF16)
    mskf_t = per.tile([128, 8], F32)
    vb_a = per.tile([128, NBG * CO], BF16, tag="vb_a")
    vb_b = per.tile([128, NBG * CO], BF16, tag="vb_b")
    vb_ab = [vb_a, vb_b]
    # Block-diag c staging for the s-pass: up to two units are in
    # flight between emit_compute and emit_spass, so 2 units x 2 halves.
    KH = K // 2
    cbd_ab = [
        per.tile([128, KH * C * 8], BF16, tag=f"cbd{h}", name=f"cbd{h}")
        for h in range(4)
    ]
    cbd_slot = {}
    cbd_ctr = [0]

    nc.sync.dma_start(out=sel_t[:], in_=sel_d)
    nc.sync.dma_start(out=sel4_t[:], in_=sel4_d)
    nc.sync.dma_start(out=msk_t[:], in_=msk_d)
    nc.gpsimd.dma_start(out=xp_sb[:], in_=xp_d)
    nc.scalar.copy(mskf_t[:], msk_t[:])
    # weights: write-once resident buffer, grouped DMAs on the gp queue.
    # Leading small groups shorten the ramp for bg0's first matmuls.
    if K % 8 == 0:
        wgroups = [2, 2, 4] + [8] * ((K - 8) // 8)
    else:
        wgroups = [1] * K
    k0 = 0
    for KB in wgroups:
        nc.gpsimd.dma_start(
            out=wt_sb[:, k0 * CO : (k0 + KB) * CO],
            in_=wt_d[:, k0 * CO : (k0 + KB) * CO],
        )
        k0 += KB

    def vb_broadcast_all(v32_ap, vb_dst):
        """vb_dst[:, bg] = broadcast of group bg's rows of v32 [32, CO]."""
        vbf32 = small.tile([BL, CO], BF16, tag="vbf32")
        nc.scalar.copy(vbf32[:], v32_ap)
        for bg in range(NBG):
            vbp = vbpsum.tile([128, CO], F32, tag="vbp")
            nc.tensor.matmul(
                vbp[:],
                lhsT=sel4_t[:, bg * 128 : (bg + 1) * 128],
                rhs=vbf32[:],
                start=True,
                stop=True,
            )
            nc.scalar.copy(vb_dst[:, bg * CO : (bg + 1) * CO], vbp[:])

    def vb_broadcast(bg, v_bg_ap, vb_dst):
        """broadcast v (partitions 0..7) into all 128 partitions of vb_dst."""
        vbf_bg = small.tile([8, CO], BF16, tag="vbf_bg")
        nc.scalar.copy(vbf_bg[:], v_bg_ap)
        vbp = vbpsum.tile([128, CO], F32, tag="vbp")
        nc.tensor.matmul(vbp[:], lhsT=sel_t[:], rhs=vbf_bg[:], start=True, stop=True)
        nc.scalar.copy(vb_dst[:, bg * CO : (bg + 1) * CO], vbp[:])

    # ---------------- wavefront emission ----------------
    # Per-bg chains (u -> it1 -> it2 -> it3 -> out) are independent, so
    # emission interleaves them: u(0) it1(0) u(1) it2(0) it1(1) u(2)
    # it2(1) it1(2) u(3) it2(2) it1(3) it2(3) it3(0..3).  Each engine's
    # in-order queue then matches execution time: the DVE runs the delta
    # chain back-to-back while the tensor engine computes later groups'
    # u and earlier groups' s-pass.
    s_sb = per.tile([BL, CO], F32)
    if K % 8 == 0:
        groups0 = [2, 2, 4] + [8] * ((K - 8) // 8)
    else:
        groups0 = [1] * K
    if K % 12 == 0:
        groupsN = [12] * (K // 12)
    else:
        groupsN = groups0
    KBMAX = max(max(groups0), max(groupsN))
    KC = 3 if K % 3 == 0 else 1  # u chunks per PSUM bank / scalar copy

    def emit_u(bg, mid_hook=None, s1ps=None):
        groups = groups0 if bg == 0 else groupsN
        k0 = 0
        ups = None
        for KB in groups:
            if mid_hook is not None and k0 >= K // 4:
                mid_hook()
                mid_hook = None
            xbd_t = xbdp.tile([128, KBMAX * 128], BF16, tag="xbd")
            nc.sync.dma_start(
                out=xbd_t[:, : KB * 128],
                in_=xbd_d[bg, :, k0 * 128 : (k0 + KB) * 128],
            )
            for kk in range(KB):
                k = k0 + kk
                if k % KC == 0:
                    ups = upsum.tile([128, KC * CO], F32, tag="ups")
                nc.tensor.matmul(
                    ups[:, (k % KC) * CO : (k % KC + 1) * CO],
                    lhsT=xbd_t[:, kk * 128 : (kk + 1) * 128],
                    rhs=wt_sb[:, k * CO : (k + 1) * CO],
                    start=True,
                    stop=True,
                )
                if s1ps is not None:
                    # iter-1 shortcut for ALL batches: x for every group
                    # is resident (xp_sb), so the full s1 accumulates
                    # during bg0's pass alone and it1 for every group can
                    # start the moment u(0) finishes -- later groups'
                    # deltas then wait only on their own u, not on a
                    # squash chain hanging off their u-phase tail.
                    nc.tensor.matmul(
                        s1ps[:],
                        lhsT=xp_sb[:, k * BL : (k + 1) * BL],
                        rhs=wt_sb[:, k * CO : (k + 1) * CO],
                        start=(k == 0),
                        stop=(k == K - 1),
                    )
                if k % KC == KC - 1:
                    nc.scalar.copy(
                        u1v[:, k - KC + 1 : k + 1, bg],
                        ups[:].rearrange("p (h x) -> p h x", h=KC),
                    )
            k0 += KB

    def emit_it1_all(s1ps):
        # evacuate the full s1 and squash all 32 batches in one pass
        # (base partition 0 throughout), then 4 selection matmuls fan
        # the per-group v out to all 128 partitions of vb_a.
        nc.scalar.copy(s_sb[:], s1ps[:])
        v1 = small.tile([BL, CO], F32, tag="v1")
        _squash_sg(nc, small, s_sb[:], v1[:], pre=1.0 / C, np_=BL)
        vb_broadcast_all(v1[:], vb_ab[0])

    def emit_compute(it, bg):
        """DVE delta chain + scalar softmax + gp cbd build for one unit.
        No PE ops: the s-pass is emitted separately (emit_spass) so the
        tensor-engine queue keeps later u-phases ahead of s-passes."""
        vb = vb_ab[it % 2]
        # delta for both halves first: keeps the DVE busy on mult/reduce
        # while the scalar queue catches up to the exps.
        for kh in range(2):
            ks = kh * KH
            # delta[b,r,c] = sum_o u*v (vector engine; mul at 2x,
            # reduce at its 1x ISA cap -- the kernel's pacing chain)
            tmpt = tmpp.tile([128, KH * CO], BF16, tag="tmp")
            nc.vector.tensor_tensor(
                out=tmpt[:].rearrange("p (k x) -> p k x", k=KH),
                in0=u1v[:, ks : ks + KH, bg],
                in1=vb[:, bg * CO : (bg + 1) * CO]
                .unsqueeze(1)
                .broadcast_to([128, KH, CO]),
                op=ALU.mult,
            )
            red_in = tmpt[:].rearrange("p (k c o) -> p k c o", k=KH, c=C)
            lh = logv[:, bg, ks : ks + KH]
            if it == 2:
                nc.vector.reduce_sum(out=lh, in_=red_in, axis=AX.X)
            else:
                dtm = small.tile([128, KH * C], F32, tag="dtm")
                nc.vector.reduce_sum(
                    out=dtm[:].rearrange("p (k c) -> p k c", k=KH),
                    in_=red_in,
                    axis=AX.X,
                )
                nc.vector.tensor_tensor(
                    out=lh.rearrange("p k c -> p (k c)"),
                    in0=lh.rearrange("p k c -> p (k c)"),
                    in1=dtm[:],
                    op=ALU.add,
                )
            # softmax exp early on the scalar queue
            ch = cexpv[:, bg, ks : ks + KH]
            nc.scalar.activation(ch, lh, AF.Exp)
        for kh in range(2):
            ks = kh * KH
            ch = cexpv[:, bg, ks : ks + KH]
            sume = small.tile([128, KH], F32, tag="sume")
            nc.vector.reduce_sum(out=sume[:], in_=ch, axis=AX.X)
            rs = small.tile([128, KH], F32, tag="rs")
            nc.vector.reciprocal(rs[:], sume[:])
            rsb = small.tile([128, KH], BF16, tag="rsb")
            nc.scalar.copy(rsb[:], rs[:])
            nc.vector.tensor_tensor(
                out=ch,
                in0=ch,
                in1=rsb[:].unsqueeze(2).broadcast_to([128, KH, C]),
                op=ALU.mult,
            )
            # build block-diag c = c (x) delta-mask on gpsimd
            if kh == 0:
                cbd_slot[(it, bg)] = cbd_ctr[0] % 2
                cbd_ctr[0] += 1
            cbd_t = cbd_ab[2 * cbd_slot[(it, bg)] + kh]
            cbdv = cbd_t[:].rearrange("p (k c e) -> p k c e", k=KH, c=C)
            nc.gpsimd.tensor_tensor(
                out=cbdv,
                in0=ch.unsqueeze(3).broadcast_to([128, KH, C, 8]),
                in1=msk_t[:]
                .unsqueeze(1)
                .unsqueeze(1)
                .broadcast_to([128, KH, C, 8]),
                op=ALU.mult,
            )

    def emit_spass(it, bg, defer):
        """PE s-pass + diagonal extract for one unit; the squash is
        appended to `defer` for the caller to place in the queues."""
        vb_next = vb_ab[(it + 1) % 2]
        sps = spsum.tile([80, CO], F32, tag="sps")
        for kh in range(2):
            ks = kh * KH
            cbdv = cbd_ab[2 * cbd_slot.pop((it, bg)) + kh][:].rearrange(
                "p (k c e) -> p k c e", k=KH, c=C
            ) if kh == 1 else cbd_ab[2 * cbd_slot[(it, bg)] + kh][:].rearrange(
                "p (k c e) -> p k c e", k=KH, c=C
            )
            for kk in range(KH):
                nc.tensor.matmul(
                    sps[:],
                    lhsT=cbdv[:, kk].rearrange("p c e -> p (c e)"),
                    rhs=u1v[:, ks + kk, bg],
                    start=(ks + kk == 0),
                    stop=(ks + kk == K - 1),
                )
        stmp = small.tile([80, CO], F32, tag="stmp")
        nc.scalar.copy(stmp[:], sps[:])
        # diagonal extract (c==c') via DMA, one [8,16] block per c.
        # it3 lands straight into the full-width s_sb: the final squash
        # runs once at [32, CO] after the last group's diag (per-group
        # output squashes measured worse -- their small DVE ops stall
        # against in-flight gpsimd cbd builds between delta units).
        if it == 2:
            s_bg2 = sbg2p.tile([8, CO], F32, tag="s_bg2")
            dst = s_bg2[:]
        else:
            dst = s_sb[bg * 8 : (bg + 1) * 8, :]
        qs = (nc.sync, nc.gpsimd, nc.scalar)
        for c in range(C):
            qs[c % 3].dma_start(
                out=dst[:, c * OC : (c + 1) * OC],
                in_=stmp[c * 8 : (c + 1) * 8, c * OC : (c + 1) * OC],
            )
        if it == 3:
            return

        def deferred(bg=bg, s=s_bg2, vn=vb_next):
            v_bg2 = small.tile([8, CO], F32, tag="v_bg2")
            _squash_sg(nc, small, s[:], v_bg2[:], pre=1.0)
            vb_broadcast(bg, v_bg2[:], vn)

        defer.append(deferred)

    # The wavefront.  PE queue: u(0) vb(0) u(1) vb(1) u(2) vb(2) u(3)
    # vb(3) spass2(0..3) spass3(0..3) -- u-phases stay ahead of all
    # s-passes so the per-bg routing cycle is paced by the DVE alone.
    # compute(it,bg) is emitted inside u(bg+1)'s chunk loop (mid_hook) so
    # its exps interleave with the u-copies on the scalar queue at the
    # position matching their execution time.
    s1ps = s1psum.tile([BL, CO], F32, tag="s1ps")
    emit_u(0, s1ps=s1ps)
    emit_it1_all(s1ps)
    emit_u(1, mid_hook=lambda: emit_compute(2, 0))
    emit_u(2, mid_hook=lambda: emit_compute(2, 1))
    # it2 s-passes run on the PE as soon as each unit's c lands (the PE
    # is free once the u-phases finish); their squashes are spliced
    # between later delta units on the DVE so that by the time D2(3)
    # retires, vb_b for the early groups already exists and iteration 3
    # starts immediately.
    sq2 = []
    emit_spass(2, 0, sq2)
    emit_u(3, mid_hook=lambda: emit_compute(2, 2))
    sq2[0]()
    emit_spass(2, 1, sq2)
    emit_compute(2, 3)
    emit_spass(2, 2, sq2)
    sq2[1]()
    sq2[2]()
    emit_spass(2, 3, sq2)
    emit_compute(3, 0)
    sq2[3]()
    emit_spass(3, 0, None)
    for bg in range(1, NBG):
        emit_compute(3, bg)
        emit_spass(3, bg, None)
    v_out = small.tile([BL, CO], F32, tag="v_out")
    _squash_sg(nc, small, s_sb[:], v_out[:], pre=1.0, np_=BL)
    nc.sync.dma_start(out=out_d, in_=v_out[:])


def build(r=None):
    """Build and compile the Bass program. Returns the compiled Bacc."""
    K = _nchunks(r)
    nc = bacc.Bacc(
        "TRN2", target_bir_lowering=False, debug=False, num_devices=NCORES
    )
    xbd_d = nc.dram_tensor(
        "xbd", [NBG, 128, K * 128], BF16, kind="ExternalInput"
    ).ap()
    wt_d = nc.dram_tensor("wt", [128, K * CO], BF16, kind="ExternalInput").ap()
    xp_d = nc.dram_tensor("xp", [128, K * BL], BF16, kind="ExternalInput").ap()
    sel_d = nc.dram_tensor("sel", [8, 128], BF16, kind="ExternalInput").ap()
    sel4_d = nc.dram_tensor(
        "sel4", [BL, NBG * 128], BF16, kind="ExternalInput"
    ).ap()
    msk_d = nc.dram_tensor("msk", [128, 8], BF16, kind="ExternalInput").ap()
    out_d = nc.dram_tensor("v_out", [BL, CO], F32, kind="ExternalOutput").ap()
    with tile.TileContext(nc) as tc, ExitStack() as ctx:
        _body(ctx, tc, xbd_d, wt_d, xp_d, sel_d, sel4_d, msk_d, out_d, K)
    nc.compile()
    return nc


def make_inputs(x, weights, r=None):
    """Host-side marshalling: shard x over cores, rearrange to bf16 tiles."""
    K = _nchunks(r)
    r_full = K * G
    W = np.asarray(weights, dtype=np.float32)[0][:r_full]  # [R, C, IC, OC]
    wt = (
        W.reshape(K, G, C, IC, OC)
        .transpose(0, 1, 3, 2, 4)
        .reshape(K, 128, CO)
        .transpose(1, 0, 2)
        .reshape(128, K * CO)
        .astype(NPBF)
    )
    sel = np.zeros((8, 128), dtype=np.float32)
    bi = np.arange(8)
    gi = np.arange(G)
    sel[bi[:, None], bi[:, None] * G + gi[None, :]] = 1.0
    sel = sel.astype(NPBF)
    sel4 = np.zeros((NBG, BL, 128), dtype=np.float32)
    mi = np.arange(128)
    for bg in range(NBG):
        sel4[bg, bg * 8 + mi // G, mi] = 1.0
    sel4 = sel4.transpose(1, 0, 2).reshape(BL, NBG * 128).astype(NPBF)
    msk = np.zeros((128, 8), dtype=np.float32)
    pi = np.arange(128)
    msk[pi, pi // G] = 1.0
    msk = msk.astype(NPBF)

    in_maps = []
    xf = np.asarray(x, dtype=np.float32)[:, :r_full]
    for core in range(NCORES):
        xl = xf[core * BL : (core + 1) * BL]  # [BL, R, IC]
        xr = xl.transpose(1, 2, 0).reshape(K, G, IC, BL)  # [K, g, i, b]
        xp = (
            xr.reshape(K, 128, BL).transpose(1, 0, 2).reshape(128, K * BL)
        ).astype(NPBF)
        xrg = xr.reshape(K, G, IC, NBG, 8)
        xbd6 = np.zeros((K, G, IC, NBG, 8, G), dtype=np.float32)
        for g in range(G):
            xbd6[:, g, :, :, :, g] = xrg[:, g]
        xbd = (
            xbd6.reshape(K, 128, NBG, 128)
            .transpose(2, 1, 0, 3)
            .reshape(NBG, 128, K * 128)
            .astype(NPBF)
        )
        in_maps.append(
            {"xbd": xbd, "wt": wt, "xp": xp, "sel": sel, "sel4": sel4,
             "msk": msk}
        )
    return in_maps


_CACHE = {}


def kernel(x, weights):
    if "nc" not in _CACHE:
        _CACHE["nc"] = build()
    nc = _CACHE["nc"]
    in_maps = make_inputs(x, weights)
    res = run_bass_kernel_spmd(nc, in_maps, core_ids=list(range(NCORES)))
    outs = [res.results[i]["v_out"].reshape(BL, C, OC) for i in range(NCORES)]
    return np.concatenate(outs, axis=0)
